# revision 21
# baseline (speedup 1.0000x reference)
"""Distributed Bass kernel for nn_AttentionLayer (2-branch GAT-style layer).

Row-shard over 8 NeuronCores (512 rows each). All per-row tensors are kept
in "transposed" layout on chip (feature/column axis on SBUF partitions) so
that the masked softmax feeds the PE attention matmuls without transposes:

  e_b^T[k, i] = lrelu(s1_b[i] + s2_b[k])                (k on partitions)
  z = e + (mask01 - 1)*BIG ; p = exp(z)                 (exp underflow -> exact 0)
  out_b^T[f, i] = sum_k Wh_b[k, f] * p[k, i]            (PE, bf16)

adj2^T is computed on PE in bf16 (exact: adj is 0/1, accumulation in f32):
  adj2^T[k, i] = sum_t adj_full[t, k] * adjT_shard[t, i]

Collectives: AllGather of s2 vectors + Wh (natural layout, bf16) + adj (bf16);
BatchNorm batch stats via a tiny AllReduce. No row-max subtraction in softmax
(values are small; matches reference to ~1e-6 rel).
"""

import sys
import numpy as np

for _p in ("/opt/trn_rl_repo", "/opt/trn_rl_repo/concourse"):
    if _p not in sys.path:
        sys.path.insert(0, _p)

import ml_dtypes

N = 4096
M_CORES = 8
R = N // M_CORES          # 512 rows per core
IN_F = 512
HALF = IN_F // 2          # 256
F = 64
P = 128                   # partitions
NT = N // P               # 32 column tiles
RT = R // P               # 4 row tiles per core
G = 4                     # adj2 k-tiles per psum group
NG = NT // G              # 8 groups
ALPHA = 0.2
EPS = 1e-5
BIG = 9e15
INV_N = 1.0 / N

_CACHED = {}


def build_nc():
    from concourse import bacc, tile, mybir

    f32 = mybir.dt.float32
    bf16 = mybir.dt.bfloat16
    Alu = mybir.AluOpType
    Act = mybir.ActivationFunctionType

    nc = bacc.Bacc("TRN2", target_bir_lowering=False, debug=False,
                   num_devices=M_CORES)

    hT_p = nc.declare_dram_parameter("hT", [IN_F, R], f32, isOutput=False)
    adjb_p = nc.declare_dram_parameter("adjb", [R, N], bf16, isOutput=False)
    adjbT_p = nc.declare_dram_parameter("adjbT", [N, R], bf16, isOutput=False)
    dT_p = nc.declare_dram_parameter("dT", [N, R], bf16, isOutput=False)
    W_p = nc.declare_dram_parameter("W12", [HALF, 2 * F], f32, isOutput=False)
    a_p = nc.declare_dram_parameter("a", [F, 2], f32, isOutput=False)
    gb_p = nc.declare_dram_parameter("gb", [2 * F, 2], f32, isOutput=False)
    id_p = nc.declare_dram_parameter("ident", [P, P], f32, isOutput=False)
    out_p = nc.declare_dram_parameter("out", [R, 2 * F], f32, isOutput=True)

    RG = [list(range(M_CORES))]

    with tile.TileContext(nc) as tc:
        with (
            tc.tile_pool(name="sb", bufs=1) as sb,
            tc.tile_pool(name="sbt", bufs=3) as sbt,
            tc.tile_pool(name="psA", bufs=1, space="PSUM") as psA,
            tc.tile_pool(name="psB", bufs=4, space="PSUM") as psB,
            tc.tile_pool(name="dram", bufs=1, space="DRAM") as dram,
        ):
            # ---- persistent loads ----
            ident = sb.tile([P, P], f32)
            nc.sync.dma_start(ident[:], id_p[:])
            a_sb = sb.tile([F, 2], f32)
            nc.sync.dma_start(a_sb[:], a_p[:])
            gb_sb = sb.tile([2 * F, 2], f32)
            nc.sync.dma_start(gb_sb[:], gb_p[:])
            W_sb = []
            for t in range(2):
                w = sb.tile([P, 2 * F], f32, tag=f"w{t}")
                nc.sync.dma_start(w[:], W_p[P * t:P * (t + 1), :])
                W_sb.append(w)
            hT_sb = []
            for t in range(RT):
                ht = sb.tile([P, R], f32, tag=f"ht{t}")
                nc.sync.dma_start(ht[:], hT_p[P * t:P * (t + 1), :])
                hT_sb.append(ht)
            adjT_sb = []
            for t in range(NT):
                at = sb.tile([P, R], bf16, tag=f"adjT{t}")
                nc.sync.dma_start(at[:], adjbT_p[P * t:P * (t + 1), :])
                adjT_sb.append(at)

            ones1 = sb.tile([1, P], f32)
            nc.vector.memset(ones1[:], 1.0)
            onesb = sb.tile([P, 1], bf16)
            nc.vector.memset(onesb[:], 1.0)

            # ---- adj bf16 bounce for AllGather (DRAM->DRAM) ----
            adj_in = dram.tile([R, N], bf16)
            nc.sync.dma_start(adj_in[:], adjb_p[:])

            # ---- Wh^T = W^T @ h^T  (psum [128, 512]: b1 rows 0:64, b2 64:128)
            whT_ps = psA.tile([P, R], f32, tag="acc")
            for b in range(2):
                for t in range(2):
                    nc.tensor.matmul(
                        whT_ps[F * b:F * (b + 1), :],
                        W_sb[t][:, F * b:F * (b + 1)],
                        hT_sb[2 * b + t][:],
                        start=(t == 0), stop=(t == 1),
                    )
            whT_sb = sb.tile([P, R], f32)
            nc.vector.tensor_copy(whT_sb[:], whT_ps[:])
            # base-partition-0 copy of Wh2^T (PE shift via identity)
            wh2_ps = psB.tile([F, R], f32, tag="tmp")
            nc.tensor.matmul(wh2_ps[:], ident[F:P, F:P], whT_sb[F:P, :],
                             start=True, stop=True)
            whT2_sb = sb.tile([F, R], f32)
            nc.vector.tensor_copy(whT2_sb[:], wh2_ps[:])
            whT_b = [whT_sb, whT2_sb]

            # ---- s vectors: s{1,2}_b[i] = sum_f a_half[f] * WhT_b[f, i]
            s1_sb = []
            s2own = []
            for b in range(2):
                rhs = whT_b[b][0:F, :]
                for half in range(2):
                    sv = psB.tile([1, R], f32, tag="tmp")
                    nc.tensor.matmul(sv[:], a_sb[:, half:half + 1], rhs,
                                     start=True, stop=True)
                    dst = sb.tile([1, R], f32, tag=f"s{half}_{b}")
                    nc.vector.tensor_copy(dst[:], sv[:])
                    (s1_sb if half == 0 else s2own).append(dst)

            # ---- Wh natural (bf16) for the AllGather ----
            wh_in = dram.tile([R, 2 * F], bf16)
            for q in range(RT):
                whn = sbt.tile([P, 2 * F], bf16, tag="whn")
                for b in range(2):
                    tp = psB.tile([P, F], f32, tag="tmp")
                    nc.tensor.transpose(
                        tp[:],
                        whT_b[b][0:F, P * q:P * (q + 1)],
                        ident[0:F, 0:F],
                    )
                    nc.vector.tensor_copy(whn[:, F * b:F * (b + 1)], tp[:])
                nc.sync.dma_start(wh_in[P * q:P * (q + 1), :], whn[:])

            s_in = dram.tile([2, R], f32)
            nc.sync.dma_start(s_in[0:1, :], s2own[0][:])
            nc.sync.dma_start(s_in[1:2, :], s2own[1][:])

            # ---- collectives (queue order: small ones first) ----
            s_full = dram.tile([2 * M_CORES, R], f32, addr_space="Shared")
            nc.gpsimd.collective_compute(
                "AllGather", mybir.AluOpType.bypass, replica_groups=RG,
                ins=[s_in[:].opt()], outs=[s_full[:].opt()])
            wh_full = dram.tile([N, 2 * F], bf16, addr_space="Shared")
            nc.gpsimd.collective_compute(
                "AllGather", mybir.AluOpType.bypass, replica_groups=RG,
                ins=[wh_in[:].opt()], outs=[wh_full[:].opt()])
            adj_full = dram.tile([N, N], bf16, addr_space="Shared")
            nc.gpsimd.collective_compute(
                "AllGather", mybir.AluOpType.bypass, replica_groups=RG,
                ins=[adj_in[:].opt()], outs=[adj_full[:].opt()])

            # ---- gathered Wh -> SBUF (natural [k, 2F], bf16) ----
            whf_sb = []
            for t in range(NT):
                wf = sbt.tile([P, 2 * F], bf16, tag=f"whf{t}", bufs=1)
                nc.sync.dma_start(wf[:], wh_full[P * t:P * (t + 1), :])
                whf_sb.append(wf)

            # ---- gathered s2 -> per-partition layout [p, r, q] (k = 512r+128q+p)
            s2_sb = []
            for b in range(2):
                s2b = sb.tile([P, M_CORES, RT], f32, tag=f"s2_{b}")
                for r in range(M_CORES):
                    src = s_full[2 * r + b].rearrange("(q p) -> p q", p=P)
                    nc.sync.dma_start(s2b[:, r, :], src)
                s2_sb.append(s2b)

            # ---- s1 broadcast across partitions (PE outer-product with ones)
            s1bc = []
            for b in range(2):
                bc = psB.tile([P, R], f32, tag="tmp")
                nc.tensor.matmul(bc[:], ones1[:], s1_sb[b][:],
                                 start=True, stop=True)
                s1b = sb.tile([P, R], f32, tag=f"s1bc{b}")
                nc.vector.tensor_copy(s1b[:], bc[:])
                s1bc.append(s1b)

            # ---- accumulators ----
            accT = psA.tile([P, R], f32, tag="acc")     # [0:64] b1, [64:128] b2
            sum_1 = psA.tile([1, R], f32, tag="sum1", name="sum_1")
            sum_2 = psA.tile([1, R], f32, tag="sum2", name="sum_2")
            sums = [sum_1, sum_2]

            def softmax_tile(b, kt, mask_done_ap):
                """mask_done_ap: f32 [P, R] with (mask01-1) in {-1, 0}."""
                u = sbt.tile([P, R], f32, tag="u")
                nc.vector.tensor_scalar(
                    u[:], s1bc[b][:], s2_sb[b][:, kt // RT, kt % RT:kt % RT + 1],
                    None, op0=Alu.add)
                e = sbt.tile([P, R], f32, tag="e")
                nc.vector.scalar_tensor_tensor(
                    e[:], u[:], ALPHA, u[:], op0=Alu.mult, op1=Alu.max)
                z = sbt.tile([P, R], f32, tag="z")
                nc.vector.scalar_tensor_tensor(
                    z[:], mask_done_ap, BIG, e[:], op0=Alu.mult, op1=Alu.add)
                pt = sbt.tile([P, R], bf16, tag="pt")
                nc.scalar.activation(pt[:], z[:], Act.Exp)
                nc.tensor.matmul(sums[b][:], onesb[:], pt[:],
                                 start=(kt == 0), stop=(kt == NT - 1))
                nc.tensor.matmul(accT[F * b:F * (b + 1), :],
                                 whf_sb[kt][:, F * b:F * (b + 1)], pt[:],
                                 start=(kt == 0), stop=(kt == NT - 1))

            # ---- branch 1: mask = adjT shard (0/1) ----
            for kt in range(NT):
                m1 = sbt.tile([P, R], f32, tag="m")
                nc.vector.tensor_scalar(m1[:], adjT_sb[kt][:], -1.0, None,
                                        op0=Alu.add)
                softmax_tile(0, kt, m1[:])

            # ---- branch 2: adj2^T = adj_full^T-blocks @ adjT shard ----
            for g in range(NG):
                cnts = [psB.tile([P, R], f32, tag="tmp", name=f"cnt{g}_{j}")
                        for j in range(G)]
                for t in range(NT):
                    af = sbt.tile([P, G * P], bf16, tag="af")
                    nc.sync.dma_start(
                        af[:], adj_full[P * t:P * (t + 1),
                                        G * P * g:G * P * (g + 1)])
                    for j in range(G):
                        nc.tensor.matmul(cnts[j][:], af[:, P * j:P * (j + 1)],
                                         adjT_sb[t][:],
                                         start=(t == 0), stop=(t == NT - 1))
                for j in range(G):
                    kt = G * g + j
                    dt_t = sbt.tile([P, R], bf16, tag="dt")
                    nc.sync.dma_start(dt_t[:], dT_p[P * kt:P * (kt + 1), :])
                    m2 = sbt.tile([P, R], f32, tag="m")
                    nc.vector.tensor_scalar(m2[:], cnts[j][:], 1.0, -1.0,
                                            op0=Alu.min, op1=Alu.add)
                    nc.vector.tensor_tensor(m2[:], m2[:], dt_t[:],
                                            op=Alu.subtract)
                    softmax_tile(1, kt, m2[:])

            # ---- epilogue: normalize, BN stats + AllReduce, BN, lrelu ----
            hpT = sb.tile([P, R], f32)
            for b in range(2):
                rc = sb.tile([1, R], f32, tag=f"rc{b}")
                nc.vector.reciprocal(rc[:], sums[b][:])
                bc = psB.tile([P, R], f32, tag="tmp")
                nc.tensor.matmul(bc[:], ones1[:], rc[:],
                                 start=True, stop=True)
                rb = sbt.tile([P, R], f32, tag="u")
                nc.vector.tensor_copy(rb[:], bc[:])
                nc.vector.tensor_mul(hpT[F * b:F * (b + 1), :],
                                     accT[F * b:F * (b + 1), :],
                                     rb[F * b:F * (b + 1), :])

            sx = sb.tile([2 * F, 2], f32)
            nc.vector.tensor_reduce(sx[:, 0:1], hpT[:],
                                    axis=mybir.AxisListType.X, op=Alu.add)
            scr = sbt.tile([P, R], bf16, tag="pt")
            nc.scalar.activation(scr[:], hpT[:], Act.Square,
                                 accum_out=sx[:, 1:2])
            stats_in = dram.tile([2 * F, 2], f32)
            nc.sync.dma_start(stats_in[:], sx[:])
            stats_out = dram.tile([2 * F, 2], f32, addr_space="Shared")
            nc.gpsimd.collective_compute(
                "AllReduce", Alu.add, replica_groups=RG,
                ins=[stats_in[:].opt()], outs=[stats_out[:].opt()])
            gst = sb.tile([2 * F, 2], f32)
            nc.sync.dma_start(gst[:], stats_out[:])

            mean = sb.tile([2 * F, 1], f32)
            nc.scalar.mul(mean[:], gst[:, 0:1], INV_N)
            ex2 = sb.tile([2 * F, 1], f32)
            nc.scalar.mul(ex2[:], gst[:, 1:2], INV_N)
            var = sb.tile([2 * F, 1], f32)
            nc.vector.scalar_tensor_tensor(var[:], mean[:], -1.0, mean[:],
                                           op0=Alu.mult, op1=Alu.mult)
            nc.vector.tensor_add(var[:], var[:], ex2[:])  # ex2 - mean^2
            nc.vector.tensor_scalar_add(var[:], var[:], EPS)
            std = sb.tile([2 * F, 1], f32)
            nc.scalar.activation(std[:], var[:], Act.Sqrt)
            rstd = sb.tile([2 * F, 1], f32)
            nc.vector.reciprocal(rstd[:], std[:])
            scale = sb.tile([2 * F, 1], f32)
            nc.vector.tensor_mul(scale[:], gb_sb[:, 0:1], rstd[:])
            nbias = sb.tile([2 * F, 1], f32)
            nc.vector.scalar_tensor_tensor(nbias[:], mean[:], -1.0, scale[:],
                                           op0=Alu.mult, op1=Alu.mult)
            nc.vector.tensor_add(nbias[:], nbias[:], gb_sb[:, 1:2])

            fin = sb.tile([P, R], f32)
            nc.scalar.activation(fin[:], hpT[:], Act.Identity,
                                 bias=nbias[:], scale=scale[:])
            finl = sb.tile([P, R], f32)
            nc.vector.scalar_tensor_tensor(finl[:], fin[:], ALPHA, fin[:],
                                           op0=Alu.mult, op1=Alu.max)

            for q in range(RT):
                tp = psB.tile([P, P], f32, tag="tmp")
                nc.tensor.transpose(tp[:], finl[:, P * q:P * (q + 1)],
                                    ident[:])
                ob = sbt.tile([P, P], f32, tag="ob")
                nc.vector.tensor_copy(ob[:], tp[:])
                nc.sync.dma_start(out_p[P * q:P * (q + 1), :], ob[:])

    nc.compile()
    return nc


def _get_nc():
    if "nc" not in _CACHED:
        _CACHED["nc"] = build_nc()
    return _CACHED["nc"]


def make_in_maps(h, adj, W1, W2, a, gamma, beta):
    h = np.asarray(h, dtype=np.float32)
    adj = np.asarray(adj, dtype=np.float32)
    W12 = np.concatenate([np.asarray(W1, np.float32),
                          np.asarray(W2, np.float32)], axis=1)
    a_flat = np.asarray(a, np.float32).reshape(2 * F)
    a_np = np.ascontiguousarray(np.stack([a_flat[:F], a_flat[F:]], axis=1))
    gb = np.stack([np.asarray(gamma, np.float32),
                   np.asarray(beta, np.float32)], axis=1)
    ident = np.eye(P, dtype=np.float32)

    adj_bf = adj.astype(ml_dtypes.bfloat16)

    in_maps = []
    for c in range(M_CORES):
        r0 = c * R
        sh = adj_bf[r0:r0 + R, :]
        dT = np.zeros((N, R), dtype=ml_dtypes.bfloat16)
        dT[np.arange(r0, r0 + R), np.arange(R)] = 1
        in_maps.append({
            "hT": np.ascontiguousarray(h[r0:r0 + R, :].T),
            "adjb": np.ascontiguousarray(sh),
            "adjbT": np.ascontiguousarray(sh.T),
            "dT": dT,
            "W12": W12,
            "a": a_np,
            "gb": gb,
            "ident": ident,
        })
    return in_maps


def kernel(h, adj, W1, W2, a, gamma, beta):
    from concourse.bass_utils import run_bass_kernel_spmd

    in_maps = make_in_maps(h, adj, W1, W2, a, gamma, beta)
    nc = _get_nc()
    res = run_bass_kernel_spmd(nc, in_maps, core_ids=list(range(M_CORES)))
    outs = [np.asarray(res.results[c]["out"]) for c in range(M_CORES)]
    return np.concatenate(outs, axis=0)


# revision 24
# speedup vs baseline: 1.6219x; 1.6219x over previous
"""Distributed Bass kernel for nn_AttentionLayer (2-branch GAT-style layer).

Row-shard over 8 NeuronCores (512 rows each). All per-row tensors are kept
in "transposed" layout on chip (feature/column axis on SBUF partitions) so
that the masked softmax feeds the PE attention matmuls without transposes:

  e_b^T[k, i] = lrelu(s1_b[i] + s2_b[k])                (k on partitions)
  z = e + (mask01 - 1)*BIG ; p = exp(z)                 (exp underflow -> exact 0)
  out_b^T[f, i] = sum_k Wh_b[k, f] * p[k, i]            (PE, bf16)

adj2^T is computed on PE in fp8 DoubleRow (exact: adj is 0/1, psum f32):
  adj2^T[k, i] = sum_t adj_full[t, k] * adjT_shard[t, i]

The adj AllGather is split into 8 column chunks so branch-2 groups pipeline
against the collective. BatchNorm batch stats via a tiny AllReduce. No
row-max subtraction in softmax (values are small, no empty rows).
"""

import sys
import numpy as np

for _p in ("/opt/trn_rl_repo", "/opt/trn_rl_repo/concourse"):
    if _p not in sys.path:
        sys.path.insert(0, _p)

import ml_dtypes

N = 4096
M_CORES = 8
R = N // M_CORES          # 512 rows per core
IN_F = 512
HALF = IN_F // 2          # 256
F = 64
P = 128                   # partitions
NT = N // P               # 32 column tiles
NTP = NT // 2             # 16 row-pair tiles (DoubleRow)
RT = R // P               # 4 row tiles per core
G = 4                     # adj2 k-tiles per psum group
NG = NT // G              # 8 groups == adj AG chunks
ALPHA = 0.2
EPS = 1e-5
BIG = 9e15
INV_N = 1.0 / N

_CACHED = {}


def build_nc():
    from concourse import bacc, tile, mybir

    f32 = mybir.dt.float32
    bf16 = mybir.dt.bfloat16
    fp8 = mybir.dt.float8e4
    Alu = mybir.AluOpType
    Act = mybir.ActivationFunctionType
    DR = mybir.MatmulPerfMode.DoubleRow

    nc = bacc.Bacc("TRN2", target_bir_lowering=False, debug=False,
                   num_devices=M_CORES)

    hT_p = nc.declare_dram_parameter("hT", [IN_F, R], f32, isOutput=False)
    adjb8_p = nc.declare_dram_parameter("adjb8", [R, N], fp8, isOutput=False)
    adjbT_p = nc.declare_dram_parameter("adjbT", [N, R], fp8, isOutput=False)
    dT_p = nc.declare_dram_parameter("dT", [N, R], bf16, isOutput=False)
    W_p = nc.declare_dram_parameter("W12", [HALF, 2 * F], f32, isOutput=False)
    a_p = nc.declare_dram_parameter("a", [F, 2], f32, isOutput=False)
    gb_p = nc.declare_dram_parameter("gb", [2 * F, 2], f32, isOutput=False)
    id_p = nc.declare_dram_parameter("ident", [P, P], f32, isOutput=False)
    out_p = nc.declare_dram_parameter("out", [R, 2 * F], f32, isOutput=True)

    RG = [list(range(M_CORES))]

    with tile.TileContext(nc) as tc:
        with (
            tc.tile_pool(name="sb", bufs=1) as sb,
            tc.tile_pool(name="sbt", bufs=3) as sbt,
            tc.tile_pool(name="psA", bufs=1, space="PSUM") as psA,
            tc.tile_pool(name="psB", bufs=5, space="PSUM") as psB,
            tc.tile_pool(name="dram", bufs=1, space="DRAM") as dram,
        ):
            # ---- small persistent loads (sync queue; gate the Wh compute)
            ident = sb.tile([P, P], f32)
            nc.sync.dma_start(ident[:], id_p[:])
            a_sb = sb.tile([F, 2], f32)
            nc.sync.dma_start(a_sb[:], a_p[:])
            gb_sb = sb.tile([2 * F, 2], f32)
            nc.sync.dma_start(gb_sb[:], gb_p[:])
            W_sb = []
            for t in range(2):
                w = sb.tile([P, 2 * F], f32, tag=f"w{t}")
                nc.sync.dma_start(w[:], W_p[P * t:P * (t + 1), :])
                W_sb.append(w)
            hT_sb = []
            for t in range(RT):
                ht = sb.tile([P, R], f32, tag=f"ht{t}")
                nc.sync.dma_start(ht[:], hT_p[P * t:P * (t + 1), :])
                hT_sb.append(ht)

            ones1 = sb.tile([1, P], f32)
            nc.vector.memset(ones1[:], 1.0)
            onesb = sb.tile([P, 1], bf16)
            nc.vector.memset(onesb[:], 1.0)

            # ---- adj fp8 chunk bounces for chunked AllGather (gpsimd queue)
            adj_in = []
            for c in range(NG):
                ai = dram.tile([R, R], fp8, name=f"adj_in{c}")
                nc.gpsimd.dma_start(ai[:], adjb8_p[:, R * c:R * (c + 1)])
                adj_in.append(ai)

            # ---- Wh^T = W^T @ h^T  (psum [128, 512]: b1 rows 0:64, b2 64:128)
            whT_ps = psA.tile([P, R], f32, tag="acc")
            for b in range(2):
                for t in range(2):
                    nc.tensor.matmul(
                        whT_ps[F * b:F * (b + 1), :],
                        W_sb[t][:, F * b:F * (b + 1)],
                        hT_sb[2 * b + t][:],
                        start=(t == 0), stop=(t == 1),
                    )
            whT_sb = sb.tile([P, R], f32)
            nc.vector.tensor_copy(whT_sb[:], whT_ps[:])
            # base-partition-0 copy of Wh2^T (PE shift via identity)
            wh2_ps = psB.tile([F, R], f32, tag="tmp")
            nc.tensor.matmul(wh2_ps[:], ident[F:P, F:P], whT_sb[F:P, :],
                             start=True, stop=True)
            whT2_sb = sb.tile([F, R], f32)
            nc.vector.tensor_copy(whT2_sb[:], wh2_ps[:])
            whT_b = [whT_sb, whT2_sb]

            # ---- s vectors: s{1,2}_b[i] = sum_f a_half[f] * WhT_b[f, i]
            s1_sb = []
            s2own = []
            for b in range(2):
                rhs = whT_b[b][0:F, :]
                for half in range(2):
                    sv = psB.tile([1, R], f32, tag="tmp")
                    nc.tensor.matmul(sv[:], a_sb[:, half:half + 1], rhs,
                                     start=True, stop=True)
                    dst = sb.tile([1, R], f32, tag=f"s{half}_{b}")
                    nc.vector.tensor_copy(dst[:], sv[:])
                    (s1_sb if half == 0 else s2own).append(dst)

            # ---- Wh natural (bf16) for the AllGather ----
            wh_in = dram.tile([R, 2 * F], bf16)
            for q in range(RT):
                whn = sbt.tile([P, 2 * F], bf16, tag="whn")
                for b in range(2):
                    tp = psB.tile([P, F], f32, tag="tmp")
                    nc.tensor.transpose(
                        tp[:],
                        whT_b[b][0:F, P * q:P * (q + 1)],
                        ident[0:F, 0:F],
                    )
                    nc.vector.tensor_copy(whn[:, F * b:F * (b + 1)], tp[:])
                nc.sync.dma_start(wh_in[P * q:P * (q + 1), :], whn[:])

            s_in = dram.tile([2, R], f32)
            nc.sync.dma_start(s_in[0:1, :], s2own[0][:])
            nc.sync.dma_start(s_in[1:2, :], s2own[1][:])

            # ---- collectives (queue order: small first, then adj chunks)
            s_full = dram.tile([2 * M_CORES, R], f32, addr_space="Shared")
            nc.gpsimd.collective_compute(
                "AllGather", Alu.bypass, replica_groups=RG,
                ins=[s_in[:].opt()], outs=[s_full[:].opt()])
            wh_full = dram.tile([N, 2 * F], bf16, addr_space="Shared")
            nc.gpsimd.collective_compute(
                "AllGather", Alu.bypass, replica_groups=RG,
                ins=[wh_in[:].opt()], outs=[wh_full[:].opt()])
            adj_chunk = []
            for c in range(NG):
                ac = dram.tile([N, R], fp8, addr_space="Shared",
                               name=f"adj_chunk{c}")
                nc.gpsimd.collective_compute(
                    "AllGather", Alu.bypass, replica_groups=RG,
                    ins=[adj_in[c][:].opt()], outs=[ac[:].opt()])
                adj_chunk.append(ac)

            # ---- transposed adj shard (fp8, DoubleRow pairing) ----
            adjT_sb = []
            for t in range(NTP):
                at = sb.tile([P, 2, R], fp8, tag=f"adjT{t}")
                src = adjbT_p[2 * P * t:2 * P * (t + 1), :]
                nc.sync.dma_start(at[:], src.rearrange("(s p) i -> p s i",
                                                       p=P))
                adjT_sb.append(at)

            # ---- gathered Wh -> SBUF (natural [k, 2F], bf16) ----
            whf_sb = []
            for t in range(NT):
                wf = sbt.tile([P, 2 * F], bf16, tag=f"whf{t}", bufs=1)
                nc.sync.dma_start(wf[:], wh_full[P * t:P * (t + 1), :])
                whf_sb.append(wf)

            # ---- gathered s2 -> per-partition layout [p, r, q] (k = 512r+128q+p)
            s2_sb = []
            for b in range(2):
                s2b = sb.tile([P, M_CORES, RT], f32, tag=f"s2_{b}")
                for r in range(M_CORES):
                    src = s_full[2 * r + b].rearrange("(q p) -> p q", p=P)
                    nc.sync.dma_start(s2b[:, r, :], src)
                s2_sb.append(s2b)

            # ---- s1 broadcast across partitions (PE outer-product with ones)
            s1bc = []
            for b in range(2):
                bc = psB.tile([P, R], f32, tag="tmp")
                nc.tensor.matmul(bc[:], ones1[:], s1_sb[b][:],
                                 start=True, stop=True)
                s1b = sb.tile([P, R], f32, tag=f"s1bc{b}")
                nc.vector.tensor_copy(s1b[:], bc[:])
                s1bc.append(s1b)

            # ---- accumulators ----
            accT = psA.tile([P, R], f32, tag="acc")     # [0:64] b1, [64:128] b2
            sum_1 = psA.tile([1, R], f32, tag="sum1", name="sum_1")
            sum_2 = psA.tile([1, R], f32, tag="sum2", name="sum_2")
            sums = [sum_1, sum_2]

            def softmax_tile(b, kt, mask_done_ap):
                """mask_done_ap: f32 [P, R] with (mask01-1) in {-1, 0}."""
                u = sbt.tile([P, R], f32, tag="u")
                nc.vector.tensor_scalar(
                    u[:], s1bc[b][:], s2_sb[b][:, kt // RT, kt % RT:kt % RT + 1],
                    None, op0=Alu.add)
                e = sbt.tile([P, R], f32, tag="e")
                nc.vector.scalar_tensor_tensor(
                    e[:], u[:], ALPHA, u[:], op0=Alu.mult, op1=Alu.max)
                z = sbt.tile([P, R], f32, tag="z")
                nc.vector.scalar_tensor_tensor(
                    z[:], mask_done_ap, BIG, e[:], op0=Alu.mult, op1=Alu.add)
                pt = sbt.tile([P, R], bf16, tag="pt", bufs=8)
                nc.scalar.activation(pt[:], z[:], Act.Exp)
                nc.tensor.matmul(sums[b][:], onesb[:], pt[:],
                                 start=(kt == 0), stop=(kt == NT - 1))
                nc.tensor.matmul(accT[F * b:F * (b + 1), :],
                                 whf_sb[kt][:, F * b:F * (b + 1)], pt[:],
                                 start=(kt == 0), stop=(kt == NT - 1))

            # ---- branch 1: mask = adjT shard (0/1) ----
            for kt in range(NT):
                m1 = sbt.tile([P, R], f32, tag="m")
                nc.vector.tensor_scalar(m1[:], adjT_sb[kt // 2][:, kt % 2, :],
                                        -1.0, None, op0=Alu.add)
                softmax_tile(0, kt, m1[:])

            # ---- branch 2: adj2^T = adj_full^T-blocks @ adjT shard (fp8 DR)
            for g in range(NG):
                cnts = [psB.tile([P, R], f32, tag="tmp", name=f"cnt{g}_{j}")
                        for j in range(G)]
                for t in range(NTP):
                    af = sbt.tile([P, 2, R], fp8, tag="af")
                    src = adj_chunk[g][2 * P * t:2 * P * (t + 1), :]
                    nc.sync.dma_start(af[:],
                                      src.rearrange("(s p) k -> p s k", p=P))
                    for j in range(G):
                        nc.tensor.matmul(cnts[j][:],
                                         af[:, :, P * j:P * (j + 1)],
                                         adjT_sb[t][:],
                                         perf_mode=DR,
                                         start=(t == 0), stop=(t == NTP - 1))
                for j in range(G):
                    kt = G * g + j
                    dt_t = sbt.tile([P, R], bf16, tag="dt")
                    nc.sync.dma_start(dt_t[:], dT_p[P * kt:P * (kt + 1), :])
                    m2 = sbt.tile([P, R], f32, tag="m")
                    nc.vector.tensor_scalar(m2[:], cnts[j][:], 1.0, -1.0,
                                            op0=Alu.min, op1=Alu.add)
                    nc.vector.tensor_tensor(m2[:], m2[:], dt_t[:],
                                            op=Alu.subtract)
                    softmax_tile(1, kt, m2[:])

            # ---- epilogue: normalize, BN stats + AllReduce, BN, lrelu ----
            hpT = sb.tile([P, R], f32)
            for b in range(2):
                rc = sb.tile([1, R], f32, tag=f"rc{b}")
                nc.vector.reciprocal(rc[:], sums[b][:])
                bc = psB.tile([P, R], f32, tag="tmp")
                nc.tensor.matmul(bc[:], ones1[:], rc[:],
                                 start=True, stop=True)
                rb = sbt.tile([P, R], f32, tag="u")
                nc.vector.tensor_copy(rb[:], bc[:])
                nc.vector.tensor_mul(hpT[F * b:F * (b + 1), :],
                                     accT[F * b:F * (b + 1), :],
                                     rb[F * b:F * (b + 1), :])

            sx = sb.tile([2 * F, 2], f32)
            nc.vector.tensor_reduce(sx[:, 0:1], hpT[:],
                                    axis=mybir.AxisListType.X, op=Alu.add)
            scr = sbt.tile([P, R], bf16, tag="pt", bufs=8)
            nc.scalar.activation(scr[:], hpT[:], Act.Square,
                                 accum_out=sx[:, 1:2])
            stats_in = dram.tile([2 * F, 2], f32)
            nc.sync.dma_start(stats_in[:], sx[:])
            stats_out = dram.tile([2 * F, 2], f32, addr_space="Shared")
            nc.gpsimd.collective_compute(
                "AllReduce", Alu.add, replica_groups=RG,
                ins=[stats_in[:].opt()], outs=[stats_out[:].opt()])
            gst = sb.tile([2 * F, 2], f32)
            nc.sync.dma_start(gst[:], stats_out[:])

            mean = sb.tile([2 * F, 1], f32)
            nc.scalar.mul(mean[:], gst[:, 0:1], INV_N)
            ex2 = sb.tile([2 * F, 1], f32)
            nc.scalar.mul(ex2[:], gst[:, 1:2], INV_N)
            var = sb.tile([2 * F, 1], f32)
            nc.vector.scalar_tensor_tensor(var[:], mean[:], -1.0, mean[:],
                                           op0=Alu.mult, op1=Alu.mult)
            nc.vector.tensor_add(var[:], var[:], ex2[:])  # ex2 - mean^2
            nc.vector.tensor_scalar_add(var[:], var[:], EPS)
            std = sb.tile([2 * F, 1], f32)
            nc.scalar.activation(std[:], var[:], Act.Sqrt)
            rstd = sb.tile([2 * F, 1], f32)
            nc.vector.reciprocal(rstd[:], std[:])
            scale = sb.tile([2 * F, 1], f32)
            nc.vector.tensor_mul(scale[:], gb_sb[:, 0:1], rstd[:])
            nbias = sb.tile([2 * F, 1], f32)
            nc.vector.scalar_tensor_tensor(nbias[:], mean[:], -1.0, scale[:],
                                           op0=Alu.mult, op1=Alu.mult)
            nc.vector.tensor_add(nbias[:], nbias[:], gb_sb[:, 1:2])

            fin = sb.tile([P, R], f32)
            nc.scalar.activation(fin[:], hpT[:], Act.Identity,
                                 bias=nbias[:], scale=scale[:])
            finl = sb.tile([P, R], f32)
            nc.vector.scalar_tensor_tensor(finl[:], fin[:], ALPHA, fin[:],
                                           op0=Alu.mult, op1=Alu.max)

            for q in range(RT):
                tp = psB.tile([P, P], f32, tag="tmp")
                nc.tensor.transpose(tp[:], finl[:, P * q:P * (q + 1)],
                                    ident[:])
                ob = sbt.tile([P, P], f32, tag="ob")
                nc.vector.tensor_copy(ob[:], tp[:])
                nc.sync.dma_start(out_p[P * q:P * (q + 1), :], ob[:])

    nc.compile()
    return nc


def _get_nc():
    if "nc" not in _CACHED:
        _CACHED["nc"] = build_nc()
    return _CACHED["nc"]


def make_in_maps(h, adj, W1, W2, a, gamma, beta):
    h = np.asarray(h, dtype=np.float32)
    adj = np.asarray(adj, dtype=np.float32)
    W12 = np.concatenate([np.asarray(W1, np.float32),
                          np.asarray(W2, np.float32)], axis=1)
    a_flat = np.asarray(a, np.float32).reshape(2 * F)
    a_np = np.ascontiguousarray(np.stack([a_flat[:F], a_flat[F:]], axis=1))
    gb = np.stack([np.asarray(gamma, np.float32),
                   np.asarray(beta, np.float32)], axis=1)
    ident = np.eye(P, dtype=np.float32)

    adj_f8 = adj.astype(ml_dtypes.float8_e4m3fn)

    in_maps = []
    for c in range(M_CORES):
        r0 = c * R
        sh = adj_f8[r0:r0 + R, :]
        dT = np.zeros((N, R), dtype=ml_dtypes.bfloat16)
        dT[np.arange(r0, r0 + R), np.arange(R)] = 1
        in_maps.append({
            "hT": np.ascontiguousarray(h[r0:r0 + R, :].T),
            "adjb8": np.ascontiguousarray(sh),
            "adjbT": np.ascontiguousarray(sh.T),
            "dT": dT,
            "W12": W12,
            "a": a_np,
            "gb": gb,
            "ident": ident,
        })
    return in_maps


def kernel(h, adj, W1, W2, a, gamma, beta):
    from concourse.bass_utils import run_bass_kernel_spmd

    in_maps = make_in_maps(h, adj, W1, W2, a, gamma, beta)
    nc = _get_nc()
    res = run_bass_kernel_spmd(nc, in_maps, core_ids=list(range(M_CORES)))
    outs = [np.asarray(res.results[c]["out"]) for c in range(M_CORES)]
    return np.concatenate(outs, axis=0)
